# revision 9
# baseline (speedup 1.0000x reference)
"""Single-head causal attention (B=4, S=4096, Dm=512, Dh=64, fp32) on 8 trn2 cores.

Sharding: 8 cores = 4 batches x 2 roles. Both roles process all 4096 queries of
their batch; the causal key-tiles (128 keys each) are split mod-4: role 0 takes
tiles {0,1} mod 4, role 1 takes {2,3} mod 4. Work per core is identical in
shape (SPMD-friendly); only the data differs. Host packs each core's key
columns contiguously, and combines partial (unnormalized) outputs +
denominators at the end (max-free softmax => partials are additive).

Device pipeline per q-block (512 queries):
  Q^T = Wq^T-chunks @ q_in^T-chunks (fp32r matmuls, PSUM accum) + bias
  per key-tile group (<=3 tiles): S^T[keys,q] = K^T-slice.T @ Q^T  (PSUM)
  P^T = exp(S^T * 1/8)  (one ACT call per group, PSUM->SBUF, fp32r out)
  diagonal tiles: P^T *= mask (DVE)
  O^T[65,q] += V_aug-tile.T @ P^T-slice  (V_aug has a ones column => row 64
  accumulates the softmax denominator)
"""

import os
import sys

sys.path.insert(0, "/opt/trn_rl_repo")

import numpy as np

import concourse.bass as bass  # noqa: F401  (registers things)
import concourse.mybir as mybir
import concourse.tile as tile
from concourse import bacc
from concourse import bass_utils

B, S, DM, DH = 4, 4096, 512, 64
QB = 512               # queries per block
NQB = S // QB          # 8 blocks
KT = 128               # keys per tile
LOCAL_KT = 16          # key tiles per core (S / KT / 2)
LOCAL_K = LOCAL_KT * KT  # 2048 local key columns
N_CORES = 8
GROUP = 3              # key tiles per scores/exp group (PSUM banks)

FP32 = mybir.dt.float32
FP32R = mybir.dt.float32r

_CACHE = {}


def _build_program():
    nc = bacc.Bacc("TRN2", target_bir_lowering=False, debug=False,
                   num_devices=N_CORES)

    qT_d = nc.dram_tensor("qT", [DM, S], FP32R, kind="ExternalInput")
    kT_d = nc.dram_tensor("kT", [DM, LOCAL_K], FP32R, kind="ExternalInput")
    vT_d = nc.dram_tensor("vT", [DM, LOCAL_K], FP32R, kind="ExternalInput")
    wT_d = nc.dram_tensor("wT", [DM, 3 * DH + 2], FP32R, kind="ExternalInput")
    bqk_d = nc.dram_tensor("bqk", [DH, 2], FP32, kind="ExternalInput")
    bvb_d = nc.dram_tensor("bvb", [KT, DH + 2], FP32, kind="ExternalInput")
    mask_d = nc.dram_tensor("mask", [KT, 2 * QB], FP32R, kind="ExternalInput")
    oT_d = nc.dram_tensor("oT", [DH + 2, S], FP32, kind="ExternalOutput")

    NCH = DM // KT  # 4 contraction chunks

    with tile.TileContext(nc) as tc:
        with tc.tile_pool(name="persist", bufs=1) as persist, \
             tc.tile_pool(name="stage", bufs=2) as stage, \
             tc.tile_pool(name="qstage", bufs=2) as qstage, \
             tc.tile_pool(name="qt", bufs=2) as qtp, \
             tc.tile_pool(name="pt", bufs=3) as ptp, \
             tc.tile_pool(name="osb", bufs=2) as osbp, \
             tc.tile_pool(name="ps_proj", bufs=1, space="PSUM") as ps_proj, \
             tc.tile_pool(name="ps_scores", bufs=2, space="PSUM") as ps_scores, \
             tc.tile_pool(name="ps_oacc", bufs=1, space="PSUM") as ps_oacc:

            # ---- constants ----
            w_sb = persist.tile([KT, NCH, 3 * DH + 2], FP32R, tag="w")
            for c in range(NCH):
                nc.sync.dma_start(out=w_sb[:, c, :],
                                    in_=wT_d.ap()[c * KT:(c + 1) * KT, :])
            bqk_sb = persist.tile([DH, 2], FP32, tag="bqk")
            nc.gpsimd.dma_start(out=bqk_sb[:], in_=bqk_d.ap())
            bvb_sb = persist.tile([KT, DH + 2], FP32, tag="bvb")
            nc.gpsimd.dma_start(out=bvb_sb[:], in_=bvb_d.ap())
            mask_sb = persist.tile([KT, 2 * QB], FP32R, tag="mask")
            nc.sync.dma_start(out=mask_sb[:], in_=mask_d.ap())

            kt_sb = persist.tile([DH, LOCAL_K], FP32R, tag="ktp")
            v_sb = persist.tile([KT, LOCAL_KT, DH + 2], FP32R, tag="vp")

            # ---- phase A: K^T and V_aug projections over local key columns --
            for kb in range(LOCAL_K // QB):  # 4 key blocks of 512
                k_stage = stage.tile([KT, NCH, QB], FP32R, tag="kst")
                v_stage = stage.tile([KT, NCH, QB], FP32R, tag="vst")
                for c in range(NCH):
                    nc.sync.dma_start(
                        out=k_stage[:, c, :],
                        in_=kT_d.ap()[c * KT:(c + 1) * KT,
                                      kb * QB:(kb + 1) * QB])
                    nc.sync.dma_start(
                        out=v_stage[:, c, :],
                        in_=vT_d.ap()[c * KT:(c + 1) * KT,
                                      kb * QB:(kb + 1) * QB])
                # K^T block: [64, 512]
                ps_k = ps_proj.tile([DH, QB], FP32, tag="pp")
                for c in range(NCH):
                    nc.tensor.matmul(ps_k[:], w_sb[:, c, DH:2 * DH],
                                     k_stage[:, c, :],
                                     start=(c == 0), stop=(c == NCH - 1))
                nc.vector.tensor_scalar_add(
                    out=kt_sb[:, kb * QB:(kb + 1) * QB], in0=ps_k[:],
                    scalar1=bqk_sb[:, 1:2])
                # V blocks: 4 x [128, 64]
                for sub in range(QB // KT):
                    t = kb * (QB // KT) + sub
                    ps_v = ps_proj.tile([KT, DH + 2], FP32, tag="pp")
                    for c in range(NCH):
                        nc.tensor.matmul(
                            ps_v[:],
                            v_stage[:, c, sub * KT:(sub + 1) * KT],
                            w_sb[:, c, 2 * DH:3 * DH + 2],
                            start=(c == 0), stop=(c == NCH - 1))
                    nc.vector.tensor_add(out=v_sb[:, t, :], in0=ps_v[:],
                                         in1=bvb_sb[:])

            # ---- phase B: attention per q-block ----
            _PH = os.environ.get("KPHASES", "FULL")
            if _PH == "A":
                nc.gpsimd.dma_start(out=oT_d.ap()[:DH, 0:LOCAL_K],
                                    in_=kt_sb[:])
            _NB = {"A": 0, "B1": 1, "B2": 2, "B4": 4, "B5": 5, "B6": 6, "B7": 7}.get(_PH, NQB)
            for qb in range(_NB):
                q_stage = qstage.tile([KT, NCH, QB], FP32R, tag="qst")
                for c in range(NCH):
                    nc.sync.dma_start(
                        out=q_stage[:, c, :],
                        in_=qT_d.ap()[c * KT:(c + 1) * KT,
                                      qb * QB:(qb + 1) * QB])
                ps_q = ps_proj.tile([DH, QB], FP32, tag="pp")
                for c in range(NCH):
                    nc.tensor.matmul(ps_q[:], w_sb[:, c, 0:DH],
                                     q_stage[:, c, :],
                                     start=(c == 0), stop=(c == NCH - 1))
                qt_sb = qtp.tile([DH, QB], FP32R, tag="qt")
                nc.vector.tensor_scalar_add(out=qt_sb[:], in0=ps_q[:],
                                            scalar1=bqk_sb[:, 0:1])

                ntk = 2 * (qb + 1)  # local key tiles for this block
                sizes = [GROUP] * (ntk // GROUP)
                if ntk % GROUP:
                    sizes.append(ntk % GROUP)
                o_ps = ps_oacc.tile([DH + 2, QB], FP32, tag="oacc")
                n_av = 0
                t0 = 0
                for g, glen in enumerate(sizes):
                    ps_s = ps_scores.tile([KT, GROUP, QB], FP32, tag="sc")
                    for i in range(glen):
                        t = t0 + i
                        nc.tensor.matmul(
                            ps_s[:, i, :],
                            kt_sb[:, t * KT:(t + 1) * KT], qt_sb[:],
                            start=True, stop=True)
                    pt = ptp.tile([KT, GROUP, QB], FP32R, tag="pt")
                    nc.scalar.activation(
                        out=pt[:, 0:glen, :], in_=ps_s[:, 0:glen, :],
                        func=mybir.ActivationFunctionType.Exp, scale=0.125)
                    for i in range(glen):
                        t = t0 + i
                        if t >= ntk - 2:  # diagonal tile -> mask
                            m = t - (ntk - 2)
                            nc.vector.tensor_mul(
                                out=pt[:, i, :], in0=pt[:, i, :],
                                in1=mask_sb[:, m * QB:(m + 1) * QB])
                    for i in range(glen):
                        t = t0 + i
                        nc.tensor.matmul(
                            o_ps[:], v_sb[:, t, :], pt[:, i, :],
                            start=(n_av == 0), stop=(n_av == ntk - 1))
                        n_av += 1
                    t0 += glen
                o_sb = osbp.tile([DH + 2, QB], FP32, tag="osb")
                nc.vector.tensor_copy(out=o_sb[:], in_=o_ps[:])
                nc.sync.dma_start(out=oT_d.ap()[:, qb * QB:(qb + 1) * QB],
                                  in_=o_sb[:])

    nc.compile()
    return nc


def _prep_inputs(q_in, k_in, v_in, Wq, bq, Wk, bk, Wv, bv):
    """Build the 8 per-core input maps (host-side, not timed)."""
    wT = np.ascontiguousarray(np.concatenate(
        [Wq.T, Wk.T, Wv.T, np.zeros((DM, 2), np.float32)],
        axis=1)).astype(np.float32)
    bqk = np.ascontiguousarray(np.stack([bq, bk], axis=1)).astype(np.float32)
    bvb = np.concatenate(
        [np.broadcast_to(bv[None, :], (KT, DH)), np.ones((KT, 1)),
         np.zeros((KT, 1))], axis=1).astype(np.float32)

    # masks: mask_m[i, j] = 1 if j >= m*128 + i  (m = 2r, 2r+1)
    ii = np.arange(KT)[:, None]
    jj = np.arange(QB)[None, :]
    masks = {}
    for r in range(2):
        m0 = (jj >= (2 * r) * KT + ii).astype(np.float32)
        m1 = (jj >= (2 * r + 1) * KT + ii).astype(np.float32)
        masks[r] = np.ascontiguousarray(np.concatenate([m0, m1], axis=1))

    # per-role local key-column index sets (mod-4 tile split)
    col_idx = {}
    for r in range(2):
        idx = []
        for t in range(S // KT // 4):  # 8 super-tiles of 4
            g0 = 4 * t + 2 * r
            idx.append(np.arange(g0 * KT, (g0 + 2) * KT))
        col_idx[r] = np.concatenate(idx)

    in_maps = []
    for b in range(B):
        qT = np.ascontiguousarray(q_in[b].T)
        kT_full = np.ascontiguousarray(k_in[b].T)
        vT_full = np.ascontiguousarray(v_in[b].T)
        for r in range(2):
            in_maps.append({
                "qT": qT,
                "kT": np.ascontiguousarray(kT_full[:, col_idx[r]]),
                "vT": np.ascontiguousarray(vT_full[:, col_idx[r]]),
                "wT": wT,
                "bqk": bqk,
                "bvb": bvb,
                "mask": masks[r],
            })
    return in_maps


def run_on_cores(inputs, trace=False, trace_kwargs=None):
    """Compile (cached), run on the 8 cores, return (results, BassKernelResults)."""
    if "nc" not in _CACHE:
        _CACHE["nc"] = _build_program()
    nc = _CACHE["nc"]
    in_maps = _prep_inputs(**inputs)
    res = bass_utils.run_bass_kernel_spmd(
        nc, in_maps, core_ids=list(range(N_CORES)), trace=trace,
        trace_kwargs=trace_kwargs or {})
    return res


def _combine(results):
    out = np.empty((B, S, DH), dtype=np.float32)
    for b in range(B):
        o0 = results[2 * b]["oT"]
        o1 = results[2 * b + 1]["oT"]
        num = o0[:DH].astype(np.float64) + o1[:DH]
        den = o0[DH].astype(np.float64) + o1[DH]
        out[b] = (num / den).T.astype(np.float32)
    return out


def kernel(**inputs):
    res = run_on_cores(inputs)
    return _combine(res.results)
